# revision 11
# baseline (speedup 1.0000x reference)
"""DeepseekV2 MoE layer on 8 Trainium2 NeuronCores (Bass/Tile, SPMD).

Strategy (generalized expert-parallel, bf16 matmuls, fp32 accumulate):
 - Host computes the MoE gate routing in numpy (bitwise-matches the jax
   reference: top-k margins are ~1e-4, far above ulp noise).
 - All work is expressed as 17 "virtual experts": 16 routed experts
   (count_e gathered tokens x 11 intermediate 128-row tiles) plus the
   shared expert (1024 tokens x 22 tiles, combine weight 1).  Each
   expert is a divisible rectangle: token-splits and i-tile-splits are
   both exact (partial outputs scatter-add on the host), so work can be
   carved into per-core cells freely.
 - A small simulated-annealing search picks a uniform slot structure
   [(C_b, k_b)] (SPMD: shapes shared by all 8 cores) plus a guillotine
   assignment of expert rectangles to the 8*B cells, minimizing padded
   PE cycles: sum_b k_b*C_b.
 - Device per slot: GEMM1 (w_gate_up tiles stationary, gathered x^T
   moving) -> SiLU*mul -> at (bf16) -> GEMM2 (at tiles stationary,
   w_down moving) accumulated over the slot's k tiles in PSUM -> scale
   by combine weight -> DMA out.  bf16 operands at full PE rate; fp32
   PSUM accumulation.
 - Host scatter-adds all slot outputs (cw pre-scaled by 2.5 for routed,
   1.0 for shared).
"""

import numpy as np
import ml_dtypes
from contextlib import ExitStack

import concourse.bacc as bacc
import concourse.tile as tile
import concourse.mybir as mybir
from concourse.bass_utils import run_bass_kernel_spmd

# problem dims (fixed by the graded problem)
T, D, I, E = 1024, 2048, 1408, 16
SI = 2 * I               # shared expert intermediate (2816)
TOP_K, N_GROUP, TOPK_GROUP = 6, 4, 2
ROUTED_SCALE = 2.5
NCORES = 8
KT = D // 128            # 16 contraction tiles
IT = I // 128            # 11 intermediate tiles (routed)
SIT = SI // 128          # 22 intermediate tiles (shared)
NT2 = D // 512           # 4 gemm2 n-tiles

f32 = mybir.dt.float32
bf16 = mybir.dt.bfloat16
np_bf16 = ml_dtypes.bfloat16
ACT_SILU = mybir.ActivationFunctionType.Silu
ACT_SIGMOID = mybir.ActivationFunctionType.Sigmoid
_SIM_SILU = False  # CoreSim lacks Silu; True emits Sigmoid + explicit mul


# ---------------------------------------------------------------- routing
def _route(x, gate_w, bias):
    """Replicates the jax reference gate in numpy f32 (decision margins are
    >=1e-4 so ulp-level differences cannot flip the top-k)."""
    logits = (x @ gate_w.T).astype(np.float32)
    scores = (1.0 / (1.0 + np.exp(-logits))).astype(np.float32)
    s_choice = scores + bias.astype(np.float32)
    grp = s_choice.reshape(T, N_GROUP, E // N_GROUP)
    group_scores = np.sort(grp, axis=2)[:, :, -2:].sum(2, dtype=np.float32)
    grp_idx = np.argsort(-group_scores, axis=1, kind="stable")[:, :TOPK_GROUP]
    gmask = np.zeros((T, N_GROUP), dtype=bool)
    gmask[np.arange(T)[:, None], grp_idx] = True
    emask = np.repeat(gmask, E // N_GROUP, axis=1)
    masked = np.where(emask, s_choice, -np.inf)
    topk_idx = np.argsort(-masked, axis=1, kind="stable")[:, :TOP_K]
    w = np.take_along_axis(scores, topk_idx, axis=1)
    w = (w / w.sum(axis=1, keepdims=True)).astype(np.float32)
    return topk_idx, w


# ------------------------------------------------------------- packing sol
def _pad16(n, lo=128):
    return max(lo, 16 * ((n + 15) // 16))


def _greedy_cover(rects, cells, variant=0, rng=None):
    """rects: [tokens, tiles, vid, tok_off, tile_off]; cells: (C, k, cid).
    Each cell cuts the best-fitting sub-rectangle from the pool."""
    pool = [list(r) for r in rects]
    placements = []
    for C, k, cid in cells:
        best_i, best_score = -1, -1.0
        for i, (t, m, v, ot, om) in enumerate(pool):
            if t <= 0 or m <= 0:
                continue
            fill = min(C, t) * min(k, m)
            bonus = 0.0
            if variant == 0:
                if 0 <= t - C < 32 or t <= C:
                    bonus = C * 0.5
                if m <= k:
                    bonus += k * 8
            elif variant == 2:
                bonus = (t * m) * 0.001
            elif variant == 3:
                bonus = fill * 0.3 * float(rng.random())
            if fill + bonus > best_score:
                best_score, best_i = fill + bonus, i
        if best_i < 0:
            continue
        t, m, v, ot, om = pool[best_i]
        ut, um = min(C, t), min(k, m)
        placements.append((cid, v, ot, ut, om, um))
        pool.pop(best_i)
        if t > ut:
            pool.append([t - ut, um, v, ot + ut, om])
        if m > um:
            pool.append([t, m - um, v, ot, om + um])
    if any(t > 0 and m > 0 for t, m, v, *_ in pool):
        return None
    return placements


def _cover(rects, slots, rng=None):
    cells = []
    for b, (C, k) in enumerate(slots):
        for c in range(NCORES):
            cells.append((C, k, (b, c)))
    for variant in (0, 1, 2):
        pl = _greedy_cover(rects, cells, variant)
        if pl is not None:
            return pl
    if rng is not None:
        for _ in range(6):
            pl = _greedy_cover(rects, cells, 3, rng)
            if pl is not None:
                return pl
    return None


def _slot_cost(C, k):
    """PE cycles for one slot: GEMM1 chunked matmuls (LDW floor ~258cyc) +
    GEMM2 (N=512, per-128-token m-tiles), in token-unit scale (/48)."""
    g1 = 2 * k * KT * sum(max(n + 8, 258) for _, n in _chunks(C))
    g2 = 4 * k * ((C + 127) // 128) * 520
    return (g1 + g2) / 48.0


_WARM_STARTS = [
    [(448, 11), (384, 11), (240, 11), (128, 3), (128, 2)],
    [(512, 11), (384, 11), (240, 11), (128, 3)],
    [(432, 11), (384, 11), (272, 11), (128, 3)],
    [(1024, 2), (432, 11), (352, 11), (224, 6), (128, 4)],
    [(1024, 3), (512, 11), (384, 11), (256, 11)],
]


def _solve_structure(counts, iters=8000, seed=1):
    """SA over uniform slot structures; returns (slots, placements)."""
    rng = np.random.default_rng(seed)
    rects = [[c, IT, v, 0, 0] for v, c in enumerate(counts)]
    rects.append([T, SIT, E, 0, 0])
    pen = 60.0
    feas_cache = {}

    def cover_cached(sl):
        key = tuple(sl)
        if key in feas_cache:
            return feas_cache[key]
        r = _cover(rects, sl, rng) is not None
        feas_cache[key] = r
        return r

    def ecost(sl):
        return sum(_slot_cost(C, k) for C, k in sl) + pen * len(sl)

    cur = None
    for ws in _WARM_STARTS:
        ws = sorted(ws, key=lambda s: (-s[0], -s[1]))
        if _cover(rects, ws, rng) is not None:
            if cur is None or ecost(ws) < ecost(cur):
                cur = ws
    assert cur is not None, "no feasible warm start"
    cur_cost = ecost(cur)
    best = (cur_cost, list(cur))
    T0, T1 = 400.0, 10.0
    for it in range(iters):
        temp = T0 * (T1 / T0) ** (it / iters)
        cand = [list(s) for s in cur]
        move = rng.integers(0, 5)
        if move == 0 and len(cand) > 2:
            cand.pop(rng.integers(0, len(cand)))
        elif move == 1 and len(cand) < 8:
            cand.append([int(rng.choice([128, 176, 256, 336, 352, 432,
                                         512, 704])),
                         int(rng.integers(1, 12))])
        elif move == 2:
            i = rng.integers(0, len(cand))
            cand[i][0] = max(128, cand[i][0] +
                             int(rng.choice([-64, -32, -16, 16, 32, 64])))
        elif move == 3:
            i = rng.integers(0, len(cand))
            cand[i][1] = int(min(SIT, max(1, cand[i][1] +
                                          rng.choice([-2, -1, 1, 2]))))
        else:
            if len(cand) >= 2:
                i, j = rng.choice(len(cand), 2, replace=False)
                d = int(rng.integers(1, 3))
                cand[i][1] = min(SIT, cand[i][1] + d)
                cand[j][1] = max(1, cand[j][1] - d)
        cand = sorted([(int(_pad16(C)), int(k)) for C, k in cand],
                      key=lambda s: (-s[0], -s[1]))
        cc = ecost(cand)
        if cc >= cur_cost and rng.random() >= np.exp((cur_cost - cc) / temp):
            continue
        if not cover_cached(cand):
            continue
        cur, cur_cost = cand, cc
        if cc < best[0]:
            best = (cc, list(cand))
    slots = best[1]
    return slots, _cover(rects, slots, np.random.default_rng(12345))


# ------------------------------------------------------------ host packing
def _chunks(c):
    """Split capacity c into GEMM1 moving-dim chunks of <=512."""
    n = (c + 511) // 512
    base = 16 * ((c // n + 15) // 16)
    out, off = [], 0
    for i in range(n - 1):
        out.append((off, base))
        off += base
    out.append((off, c - off))
    return out


def _pack_wgu(w, it_cnt):
    """w: [2*ic, D] rows (gate block then up block, ic=128*it_cnt rows each)
    -> [2*it_cnt, 128, KT, 128], gate/up 128-row tiles interleaved."""
    ic = 128 * it_cnt
    g = w[:ic].reshape(it_cnt, 128, D)
    u = w[ic:].reshape(it_cnt, 128, D)
    inter = np.stack([g, u], axis=1).reshape(2 * it_cnt * 128, D)
    t = inter.T.reshape(KT, 128, 2 * it_cnt, 128).transpose(2, 1, 0, 3)
    return np.ascontiguousarray(t, dtype=np_bf16)


def _pack_wd(wdT, it_cnt):
    """wdT: [128*it_cnt, D] (= w_down^T rows) -> [NT2, 128, it_cnt, 512]."""
    t = wdT.reshape(it_cnt, 128, NT2, 512).transpose(2, 1, 0, 3)
    return np.ascontiguousarray(t, dtype=np_bf16)


def _pack_xT(xs, cap):
    """xs: [n, D] token rows -> [128, KT, cap] bf16 (x^T k-tiles, padded)."""
    out = np.zeros((128, KT, cap), dtype=np_bf16)
    n = xs.shape[0]
    out[:, :, :n] = xs.T.reshape(KT, 128, n).transpose(1, 0, 2).astype(np_bf16)
    return out


# ------------------------------------------------------------ device build
def _build(slots):
    nc = bacc.Bacc("TRN2", target_bir_lowering=False, debug=False,
                   num_devices=NCORES)
    dr = {}
    for b, (C, k) in enumerate(slots):
        n_mt = (C + 127) // 128
        dr[f"xg{b}"] = nc.dram_tensor(f"xg{b}", [128, KT, C], bf16,
                                      kind="ExternalInput")
        dr[f"wgu{b}"] = nc.dram_tensor(f"wgu{b}", [2 * k, 128, KT, 128],
                                       bf16, kind="ExternalInput")
        dr[f"wd{b}"] = nc.dram_tensor(f"wd{b}", [NT2, 128, k, 512], bf16,
                                      kind="ExternalInput")
        dr[f"cw{b}"] = nc.dram_tensor(f"cw{b}", [n_mt, 128], f32,
                                      kind="ExternalInput")
        dr[f"y{b}"] = nc.dram_tensor(f"y{b}", [C, D], f32,
                                     kind="ExternalOutput")

    with tile.TileContext(nc) as tc, ExitStack() as ctx:
        sb = ctx.enter_context(tc.tile_pool(name="sb", bufs=1))
        ps = ctx.enter_context(tc.tile_pool(name="ps", bufs=1, space="PSUM"))

        def ffn(b, cap, k):
            chunks = _chunks(cap)
            mtl = [(r, min(128, cap - r)) for r in range(0, cap, 128)]
            xg_d, wgu_d, wd_d, cw_d, out_d = (dr[f"xg{b}"], dr[f"wgu{b}"],
                                              dr[f"wd{b}"], dr[f"cw{b}"],
                                              dr[f"y{b}"])
            xg = sb.tile([128, KT, cap], bf16, tag=f"xbuf{b}", bufs=1,
                         name=f"xg_{b}")
            for kk in range(KT):
                nc.gpsimd.dma_start(xg[:, kk, :], xg_d.ap()[:, kk, :])
            cw = sb.tile([128, len(mtl)], f32, tag="cw", bufs=2,
                         name=f"cw_{b}")
            for m in range(len(mtl)):
                nc.gpsimd.dma_start(cw[:, m:m + 1], cw_d.ap()[m].unsqueeze(1))
            # GEMM1 + silu*mul -> at (A^T, [i-part, tokens], bf16)
            at = sb.tile([128, k, cap], bf16, tag=f"at{b}", bufs=1,
                         name=f"at_{b}")
            for t in range(k):
                pair = []
                for par in (0, 1):
                    wgu = sb.tile([128, KT, 128], bf16, tag="wgu", bufs=6,
                                  name="wgu")
                    nc.sync.dma_start(wgu[:], wgu_d.ap()[2 * t + par])
                    row = []
                    for ci, (off, n) in enumerate(chunks):
                        p = ps.tile([128, n], f32, tag=f"ps{par}", bufs=3,
                                    name=f"ps{par}")
                        for kk in range(KT):
                            nc.tensor.matmul(p[:], wgu[:, kk, :],
                                             xg[:, kk, off:off + n],
                                             start=(kk == 0),
                                             stop=(kk == KT - 1))
                        row.append(p)
                    pair.append(row)
                for ci, (off, n) in enumerate(chunks):
                    tmp = sb.tile([128, n], f32, tag="tmp", bufs=3, name="tmp")
                    if _SIM_SILU:
                        nc.scalar.activation(tmp[:], pair[0][ci][:],
                                             ACT_SIGMOID)
                        nc.vector.tensor_mul(tmp[:], tmp[:], pair[0][ci][:])
                    else:
                        nc.scalar.activation(tmp[:], pair[0][ci][:], ACT_SILU)
                    nc.vector.tensor_mul(at[:, t, off:off + n], tmp[:],
                                         pair[1][ci][:])
            # GEMM2: accumulate the slot's k tiles in PSUM per m-tile
            for nt in range(NT2):
                wd = sb.tile([128, k, 512], bf16, tag="wd", bufs=4, name="wd")
                nc.sync.dma_start(wd[:], wd_d.ap()[nt])
                for mi, (r0, p_) in enumerate(mtl):
                    yp = ps.tile([128, 512], f32, tag="psy", bufs=2, name="yp")
                    for kk in range(k):
                        nc.tensor.matmul(yp[:p_, :], at[:, kk, r0:r0 + p_],
                                         wd[:, kk, :], start=(kk == 0),
                                         stop=(kk == k - 1))
                    ysb = sb.tile([128, 512], f32, tag="ysb", bufs=3,
                                  name="ysb")
                    nc.vector.tensor_scalar_mul(ysb[:p_, :], yp[:p_, :],
                                                cw[:p_, mi:mi + 1])
                    nc.scalar.dma_start(
                        out_d.ap()[r0:r0 + p_, nt * 512:(nt + 1) * 512],
                        ysb[:p_, :])

        for b in sorted(range(len(slots)), key=lambda i: slots[i][0]):
            ffn(b, slots[b][0], slots[b][1])

    nc.compile()
    return nc


# ----------------------------------------------------------------- kernel
def kernel(x, gate_w, bias, w_gate_up, w_down, shared_w_gate_up,
           shared_w_down, _trace=False):
    x = np.ascontiguousarray(x, dtype=np.float32)
    topk_idx, w = _route(x, gate_w, bias)
    cw_full = w.astype(np.float32) * np.float32(ROUTED_SCALE)

    # virtual experts: 16 routed + shared (id 16)
    toks, wts, counts = [], [], np.zeros(E + 1, dtype=np.int64)
    for e in range(E):
        tsel, ksel = np.where(topk_idx == e)
        toks.append(tsel)
        wts.append(cw_full[tsel, ksel])
        counts[e] = len(tsel)
    toks.append(np.arange(T))
    wts.append(np.ones(T, dtype=np.float32))
    counts[E] = T

    slots, placements = _solve_structure([int(c) for c in counts[:E]])
    B = len(slots)
    cellmap = {}
    for (b, c), v, ot, nt, om, nm in placements:
        cellmap[(b, c)] = (v, ot, nt, om, nm)

    def gate_rows(v, i):
        if v < E:
            return (w_gate_up[v][128 * i:128 * (i + 1)],
                    w_gate_up[v][I + 128 * i:I + 128 * (i + 1)])
        return (shared_w_gate_up[128 * i:128 * (i + 1)],
                shared_w_gate_up[SI + 128 * i:SI + 128 * (i + 1)])

    def down_rows(v, i):
        if v < E:
            return w_down[v][:, 128 * i:128 * (i + 1)].T
        return shared_w_down[:, 128 * i:128 * (i + 1)].T

    in_maps = []
    for c in range(NCORES):
        m = {}
        for b, (C, k) in enumerate(slots):
            n_mt = (C + 127) // 128
            pl = cellmap.get((b, c))
            if pl is None:
                m[f"xg{b}"] = np.zeros((128, KT, C), dtype=np_bf16)
                m[f"wgu{b}"] = np.zeros((2 * k, 128, KT, 128), dtype=np_bf16)
                m[f"wd{b}"] = np.zeros((NT2, 128, k, 512), dtype=np_bf16)
                m[f"cw{b}"] = np.zeros((n_mt, 128), dtype=np.float32)
                continue
            v, ot, ntk, om, nm = pl
            ids = toks[v][ot:ot + ntk]
            m[f"xg{b}"] = _pack_xT(x[ids], C)
            wg = np.zeros((2 * k * 128, D), dtype=np.float32)
            for j in range(nm):
                g, u = gate_rows(v, om + j)
                wg[128 * j:128 * (j + 1)] = g
                wg[128 * (k + j):128 * (k + j + 1)] = u
            m[f"wgu{b}"] = _pack_wgu(wg, k)
            wdT = np.zeros((k * 128, D), dtype=np.float32)
            for j in range(nm):
                wdT[128 * j:128 * (j + 1)] = down_rows(v, om + j)
            m[f"wd{b}"] = _pack_wd(wdT, k)
            cwv = np.zeros(n_mt * 128, dtype=np.float32)
            cwv[:ntk] = wts[v][ot:ot + ntk]
            m[f"cw{b}"] = cwv.reshape(n_mt, 128)
        in_maps.append(m)

    nc = _build(slots)
    kw = {}
    if _trace:
        kw = dict(trace=True, trace_cores=list(range(NCORES)))
    res = run_bass_kernel_spmd(nc, in_maps, core_ids=list(range(NCORES)), **kw)

    y = np.zeros((T, D), dtype=np.float32)
    for c in range(NCORES):
        for b in range(B):
            pl = cellmap.get((b, c))
            if pl is None:
                continue
            v, ot, ntk, om, nm = pl
            if nm == 0 or ntk == 0:
                continue
            ids = toks[v][ot:ot + ntk]
            y[ids] += res.results[c][f"y{b}"][:ntk]
    if _trace:
        return y, res
    return y
